# revision 50
# baseline (speedup 1.0000x reference)
"""BiMapGeo forward on 8 NeuronCores (TRN2, Bass/Tile).

P[b,o] = sum_c W[o,c]^T @ x[b,c] @ W[o,c]
  x: (256, 8, 128, 128) fp32 (symmetric in last two dims)
  W: (16, 8, 128, 64) fp32
  P: (256, 16, 64, 64) fp32

Sharding: data-parallel over batch (32 per core), W replicated.

Per-core kernel, c-granular pipeline (per group of BG=4 batches):
  for c:  mm1: M1[b,c] = x[b,c] @ W[:,c]   (bf16, stationary=x[b,c] via
                                            symmetry, moving=W as 2x512)
          evict M1 PSUM fp32 -> SBUF bf16  (DVE/Act alternating)
          mm2(c-1): P[o] += W[o,c-1]^T @ M1[b,o,c-1]  (bf16, N=256,
                    8 PSUM accumulators held across the c loop)
  mm1 is emitted 2+2 around mm2(c-1) so the PE never waits on eviction.
  mm2(c=7) of group g runs between mm1(c=0) halves of group g+1.
"""

import numpy as np
from contextlib import ExitStack

import concourse.bacc as bacc
import concourse.tile as tile
from concourse import mybir

B_TOT, HI, HO, NI, NO = 256, 8, 16, 128, 64
NCORES = 8
B = B_TOT // NCORES  # 32 batches per core
BG = 4               # batches per group
NG = B // BG         # 8 groups
OQ = HO * NO         # 1024
HIH = HI // 2        # c-half size for x staging

F32 = mybir.dt.float32
BF16 = mybir.dt.bfloat16

_NC_CACHE = {}


def build_nc(loop_iters: int = 1):
    nc = bacc.Bacc("TRN2", target_bir_lowering=False, debug=False)

    x_in = nc.dram_tensor("x", [B, HI, NI, NI], F32, kind="ExternalInput")
    w_in = nc.dram_tensor("W", [HO, HI, NI, NO], F32, kind="ExternalInput")
    p_out = nc.dram_tensor("P", [B, HO, NO, NO], F32, kind="ExternalOutput")

    with tile.TileContext(nc) as tc, ExitStack() as ctx:
        const = ctx.enter_context(tc.tile_pool(name="const", bufs=1))
        wstage = ctx.enter_context(tc.tile_pool(name="wstage", bufs=2))
        xstage = ctx.enter_context(tc.tile_pool(name="xstage", bufs=8))
        xpool = ctx.enter_context(tc.tile_pool(name="xpool", bufs=3))
        m1pool = ctx.enter_context(tc.tile_pool(name="m1pool", bufs=3))
        ppool = ctx.enter_context(tc.tile_pool(name="ppool", bufs=2))
        m1ps_pool = ctx.enter_context(tc.tile_pool(name="m1ps", bufs=2, space="PSUM"))
        pps_pool = ctx.enter_context(tc.tile_pool(name="pps", bufs=4, space="PSUM"))

        # W resident in SBUF as [j(128), c, o, q] bf16: moving operand for
        # mm1 (slices [j, 512]), stationary for mm2 (slices [i, 64]).
        w_bf = const.tile([NI, HI, HO, NO], BF16, tag="w_bf")

        def w_load(c):
            # Per (c, o-half) so mm1(c=0) only waits on a 256KB transfer.
            # Late c's ride the Pool SWDGE queue: descriptor generation on
            # the two HWDGE queues is the startup bottleneck (~1.6us per
            # half), and Pool is idle once the first x rounds are done.
            for h in range(2):
                w_st = wstage.tile([NI, HO // 2, NO], F32, tag="wst",
                                   name=f"wst{c}h{h}")
                if c >= 4:
                    q = nc.gpsimd
                elif c == 0:
                    q = nc.sync if h == 0 else nc.scalar
                else:
                    q = nc.sync if c % 2 == 0 else nc.scalar
                q.dma_start(
                    out=w_st[:],
                    in_=w_in[h * 8:(h + 1) * 8, c, :, :].transpose([1, 0, 2]),
                )
                eng = ("v", "a", "p", "v", "a", "p", "v", "a")[c]
                dst = w_bf[:, c, h * 8:(h + 1) * 8]
                if eng == "a":
                    nc.scalar.copy(dst, w_st[:])
                elif eng == "v":
                    nc.vector.tensor_copy(dst, w_st[:])
                else:
                    nc.gpsimd.tensor_copy(dst, w_st[:])

        w_load(0)

        def emit_body():
            emit_groups(nc, tc, x_in, p_out, w_bf, w_load,
                        xstage, xpool, m1pool, ppool, m1ps_pool, pps_pool)

        if loop_iters > 1:
            for c in range(1, HI):
                w_load(c)
            ET = mybir.EngineType
            with tc.For_i(0, loop_iters, 1, hint_engines=(ET.PE, ET.DVE, ET.Activation, ET.SP)):
                emit_groups(nc, tc, x_in, p_out, w_bf, None,
                            xstage, xpool, m1pool, ppool, m1ps_pool, pps_pool)
        else:
            emit_body()
    nc.finalize()
    return nc


def emit_groups(nc, tc, x_in, p_out, w_bf, w_load_rest, xstage, xpool, m1pool, ppool, m1ps_pool, pps_pool):
    # x tile per group: [j(128), b, c, i] bf16; by symmetry usable as
    # [i, b, c, j]. DMA per (b, c-half) on SP, round to bf16 on Pool.
    # h-major order so the first c-half of every batch arrives first.
    def x_load(g, fine_b0=False):
        x_t = xpool.tile([NI, BG, HI, NI], BF16, tag="xt", name=f"xt{g}")
        for h in range(2):
            for b in range(BG):
                if fine_b0 and b == 0:
                    # 2-c granularity for the first batch of group 0 so
                    # mm1(c=0,b=0) starts ~2.5us in instead of ~6.5us.
                    for s in range(2):
                        c0 = h * HIH + s * 2
                        x_sb = xstage.tile([NI, 2, NI], F32, tag="xstf",
                                           name=f"xstf{h}s{s}")
                        nc.sync.dma_start(
                            out=x_sb[:],
                            in_=x_in[g * BG, c0:c0 + 2].transpose([1, 0, 2]),
                        )
                        nc.gpsimd.tensor_copy(x_t[:, 0, c0:c0 + 2], x_sb[:])
                    continue
                x_sb = xstage.tile([NI, HIH, NI], F32, tag="xst",
                                   name=f"xst{g}b{b}h{h}")
                nc.sync.dma_start(
                    out=x_sb[:],
                    in_=x_in[g * BG + b, h * HIH:(h + 1) * HIH].transpose([1, 0, 2]),
                )
                nc.gpsimd.tensor_copy(x_t[:, b, h * HIH:(h + 1) * HIH], x_sb[:])
        return x_t

    def mm2_part(pps_tiles, c, m1_c, half):
        # P[o] accumulation for one c: 16 matmuls of N=256, two o's per
        # PSUM tile column-block via tile_position row packing; emitted in
        # a 9-MM and a 7-MM batch so they pad the two m1_ps WAR windows
        # (DVE evictions need ~1292ns, Act ~1138ns).
        # Two o-pairs share each 2KB PSUM bank. start=True marks the whole
        # bank (per touched partition range) pending-zero, so only the
        # first column block (t%2==0) starts; block 1's first write then
        # lands on pending-zero bytes and overwrites correctly.
        rng = range(0, 8) if half == 0 else range(8, 16)
        for k in rng:
            t, ph = k // 2, k % 2
            pt = pps_tiles[t // 2]
            o = 2 * t + ph
            nc.tensor.matmul(
                pt[ph * 64:(ph + 1) * 64, t % 2, :],
                w_bf[:, c, o, :],
                m1_c[:, :, o * NO:(o + 1) * NO],
                start=(c == 0 and t % 2 == 0),
                stop=(c == HI - 1),
                tile_position=(0, ph * 64),
                skip_group_check=True,
            )

    def mm1(x_t, c, b, m1_c):
        m1_ps = m1ps_pool.tile([NI, OQ], F32, tag="m1ps")
        lhsT = x_t[:, b, c, :]
        for h in range(2):
            nc.tensor.matmul(
                m1_ps[:, h * 512:(h + 1) * 512],
                lhsT,
                w_bf[:, c, h * 8:(h + 1) * 8, :],
                start=True,
                stop=True,
            )
        # One eviction copy per batch, alternating DVE/Act. Any finer split
        # (even h-aligned staggered halves) adds ~400ns effective per-copy
        # overhead and measured worse in every arrangement tried.
        if b % 2 == 0:
            nc.vector.tensor_copy(m1_c[:, b, :], m1_ps[:, :])
        else:
            nc.scalar.copy(m1_c[:, b, :], m1_ps[:, :])

    def evict_p_pair(p_sb, pps_tiles, b0, i, eng, dma_qs=None):
        # Copy accumulator pair-tile i (o-pairs 2i, 2i+1) to SBUF, then
        # one DMA per o-pair. GPSIMD can't read PSUM on HW, so these ride
        # DVE/Act; callers stage pairs across c=0/c=1 of the next group to
        # stay inside the per-c engine budgets. For the last group (p_sb
        # None) no mm2 follows, so DMA straight from PSUM and skip the
        # staging copy.
        if p_sb is not None:
            if eng == "v":
                nc.vector.tensor_copy(p_sb[:, 2 * i:2 * i + 2, :], pps_tiles[i][:])
            else:
                nc.scalar.copy(p_sb[:, 2 * i:2 * i + 2, :], pps_tiles[i][:])
        for k in range(2):
            t = 2 * i + k
            q = dma_qs[k] if dma_qs else nc.sync
            src = p_sb[:, t, :] if p_sb is not None else pps_tiles[i][:, k, :]
            q.dma_start(
                out=p_out[b0:b0 + BG, 2 * t:2 * t + 2].rearrange(
                    "b o p q -> (o p) b q"
                ),
                in_=src,
            )

    x_tiles = {0: x_load(0, fine_b0=(w_load_rest is not None))}
    if w_load_rest is not None:
        for c in range(1, HI):
            w_load_rest(c)
    prev = None  # [pps_tiles, m1_tile_of_c7, b0, g, p_sb] pending mm2(c=7) + evict

    for g in range(NG):
        b0 = g * BG
        if g + 1 < NG:
            x_tiles[g + 1] = x_load(g + 1)
        x_t = x_tiles.pop(g)

        pps_tiles = [
            pps_pool.tile([NI, 2, BG * NO], F32, tag="pps", name=f"pps{g}_{i}")
            for i in range(4)
        ]

        m1_prev = None
        for c in range(HI):
            m1_c = m1pool.tile([NI, BG, OQ], BF16, tag="m1")
            mm1(x_t, c, 0, m1_c)
            mm1(x_t, c, 1, m1_c)
            if c > 0:
                mm2_part(pps_tiles, c - 1, m1_prev, 0)
            elif prev is not None:
                mm2_part(prev[0], HI - 1, prev[1], 0)
            if c == 0 and prev is not None:
                # P eviction of the previous group, pair tiles 0,1 (o 0..3):
                # emitted here (after their stops in mm2(c7) h0) so they sit
                # between m1-evict halves in the Act FIFO without starving
                # the WAR chains.
                ppp, m1p, b0p, gp, _ = prev
                p_sb = ppool.tile([NI, HO // 2, BG * NO], F32, tag="psb",
                                  name=f"psb{gp}")
                prev[4] = p_sb
                evict_p_pair(p_sb, ppp, b0p, 0, "a")
                evict_p_pair(p_sb, ppp, b0p, 1, "a")
            elif c == 1 and prev is not None:
                ppp, m1p, b0p, gp, p_sb = prev
                evict_p_pair(p_sb, ppp, b0p, 2, "a")
                evict_p_pair(p_sb, ppp, b0p, 3, "a")
            mm1(x_t, c, 2, m1_c)
            mm1(x_t, c, 3, m1_c)
            if c > 0:
                mm2_part(pps_tiles, c - 1, m1_prev, 1)
                if c == 1:
                    prev = None
            elif prev is not None:
                mm2_part(prev[0], HI - 1, prev[1], 1)
            m1_prev = m1_c

        prev = [pps_tiles, m1_prev, b0, g, None]

    ppp, m1p, b0p, gp, _ = prev
    mm2_part(ppp, HI - 1, m1p, 0)
    mm2_part(ppp, HI - 1, m1p, 1)
    p_sb = ppool.tile([NI, HO // 2, BG * NO], F32, tag="psb", name=f"psb{gp}")
    for i in range(4):
        evict_p_pair(p_sb, ppp, b0p, i, "v" if i % 2 == 0 else "a",
                     dma_qs=(nc.sync, nc.scalar))


def kernel(x: np.ndarray, W: np.ndarray) -> np.ndarray:
    from concourse.bass_utils import run_bass_kernel_spmd

    x = np.ascontiguousarray(x, dtype=np.float32)
    W = np.ascontiguousarray(W, dtype=np.float32)

    if "nc" not in _NC_CACHE:
        _NC_CACHE["nc"] = build_nc()
    nc = _NC_CACHE["nc"]

    in_maps = [
        {"x": x[i * B : (i + 1) * B], "W": W} for i in range(NCORES)
    ]
    res = run_bass_kernel_spmd(nc, in_maps, list(range(NCORES)))
    out = np.concatenate([res.results[i]["P"] for i in range(NCORES)], axis=0)
    return out


# revision 53
# speedup vs baseline: 1.0143x; 1.0143x over previous
"""BiMapGeo forward on 8 NeuronCores (TRN2, Bass/Tile).

P[b,o] = sum_c W[o,c]^T @ x[b,c] @ W[o,c]
  x: (256, 8, 128, 128) fp32 (symmetric in last two dims)
  W: (16, 8, 128, 64) fp32
  P: (256, 16, 64, 64) fp32

Sharding: data-parallel over batch (32 per core), W replicated.

Per-core kernel, c-granular pipeline (per group of BG=4 batches):
  for c:  mm1: M1[b,c] = x[b,c] @ W[:,c]   (bf16, stationary=x[b,c] via
                                            symmetry, moving=W as 2x512)
          evict M1 PSUM fp32 -> SBUF bf16  (DVE/Act alternating)
          mm2(c-1): P[o] += W[o,c-1]^T @ M1[b,o,c-1]  (bf16, N=256,
                    8 PSUM accumulators held across the c loop)
  mm1 is emitted 2+2 around mm2(c-1) so the PE never waits on eviction.
  mm2(c=7) of group g runs between mm1(c=0) halves of group g+1.
"""

import numpy as np
from contextlib import ExitStack

import concourse.bacc as bacc
import concourse.tile as tile
from concourse import mybir

B_TOT, HI, HO, NI, NO = 256, 8, 16, 128, 64
NCORES = 8
B = B_TOT // NCORES  # 32 batches per core
BG = 4               # batches per group
NG = B // BG         # 8 groups
OQ = HO * NO         # 1024
HIH = HI // 2        # c-half size for x staging

F32 = mybir.dt.float32
BF16 = mybir.dt.bfloat16

_NC_CACHE = {}


def build_nc(loop_iters: int = 1):
    nc = bacc.Bacc("TRN2", target_bir_lowering=False, debug=False)

    x_in = nc.dram_tensor("x", [B, HI, NI, NI], F32, kind="ExternalInput")
    w_in = nc.dram_tensor("W", [HO, HI, NI, NO], F32, kind="ExternalInput")
    p_out = nc.dram_tensor("P", [B, HO, NO, NO], F32, kind="ExternalOutput")

    with tile.TileContext(nc) as tc, ExitStack() as ctx:
        const = ctx.enter_context(tc.tile_pool(name="const", bufs=1))
        wstage = ctx.enter_context(tc.tile_pool(name="wstage", bufs=2))
        xstage = ctx.enter_context(tc.tile_pool(name="xstage", bufs=8))
        xpool = ctx.enter_context(tc.tile_pool(name="xpool", bufs=3))
        m1pool = ctx.enter_context(tc.tile_pool(name="m1pool", bufs=3))
        ppool = ctx.enter_context(tc.tile_pool(name="ppool", bufs=2))
        m1ps_pool = ctx.enter_context(tc.tile_pool(name="m1ps", bufs=2, space="PSUM"))
        pps_pool = ctx.enter_context(tc.tile_pool(name="pps", bufs=4, space="PSUM"))

        # W resident in SBUF as [j(128), c, o, q] bf16: moving operand for
        # mm1 (slices [j, 512]), stationary for mm2 (slices [i, 64]).
        w_bf = const.tile([NI, HI, HO, NO], BF16, tag="w_bf")

        def w_load(c):
            # Per (c, o-half) so mm1(c=0) only waits on a 256KB transfer.
            # Late c's ride the Pool SWDGE queue: descriptor generation on
            # the two HWDGE queues is the startup bottleneck (~1.6us per
            # half), and Pool is idle once the first x rounds are done.
            for h in range(2):
                w_st = wstage.tile([NI, HO // 2, NO], F32, tag="wst",
                                   name=f"wst{c}h{h}")
                if c >= 4:
                    q = nc.gpsimd
                elif c == 0:
                    q = nc.sync if h == 0 else nc.scalar
                else:
                    q = nc.sync if c % 2 == 0 else nc.scalar
                q.dma_start(
                    out=w_st[:],
                    in_=w_in[h * 8:(h + 1) * 8, c, :, :].transpose([1, 0, 2]),
                )
                eng = ("v", "a", "p", "v", "a", "p", "v", "a")[c]
                dst = w_bf[:, c, h * 8:(h + 1) * 8]
                if eng == "a":
                    nc.scalar.copy(dst, w_st[:])
                elif eng == "v":
                    nc.vector.tensor_copy(dst, w_st[:])
                else:
                    nc.gpsimd.tensor_copy(dst, w_st[:])

        w_load(0)

        def emit_body():
            emit_groups(nc, tc, x_in, p_out, w_bf, w_load,
                        xstage, xpool, m1pool, ppool, m1ps_pool, pps_pool)

        if loop_iters > 1:
            for c in range(1, HI):
                w_load(c)
            ET = mybir.EngineType
            with tc.For_i(0, loop_iters, 1, hint_engines=(ET.PE, ET.DVE, ET.Activation, ET.SP)):
                emit_groups(nc, tc, x_in, p_out, w_bf, None,
                            xstage, xpool, m1pool, ppool, m1ps_pool, pps_pool)
        else:
            emit_body()
    nc.finalize()
    return nc


def emit_groups(nc, tc, x_in, p_out, w_bf, w_load_rest, xstage, xpool, m1pool, ppool, m1ps_pool, pps_pool):
    # x tile per group: [j(128), b, c, i] bf16; by symmetry usable as
    # [i, b, c, j]. DMA per (b, c-half) on SP, round to bf16 on Pool.
    # h-major order so the first c-half of every batch arrives first.
    def x_load(g, fine_b0=False):
        x_t = xpool.tile([NI, BG, HI, NI], BF16, tag="xt", name=f"xt{g}")
        for h in range(2):
            for b in range(BG):
                if fine_b0 and b == 0:
                    # 2-c granularity for the first batch of group 0 so
                    # mm1(c=0,b=0) starts ~2.5us in instead of ~6.5us.
                    for s in range(2):
                        c0 = h * HIH + s * 2
                        x_sb = xstage.tile([NI, 2, NI], F32, tag="xstf",
                                           name=f"xstf{h}s{s}")
                        nc.sync.dma_start(
                            out=x_sb[:],
                            in_=x_in[g * BG, c0:c0 + 2].transpose([1, 0, 2]),
                        )
                        nc.gpsimd.tensor_copy(x_t[:, 0, c0:c0 + 2], x_sb[:])
                    continue
                x_sb = xstage.tile([NI, HIH, NI], F32, tag="xst",
                                   name=f"xst{g}b{b}h{h}")
                nc.sync.dma_start(
                    out=x_sb[:],
                    in_=x_in[g * BG + b, h * HIH:(h + 1) * HIH].transpose([1, 0, 2]),
                )
                nc.gpsimd.tensor_copy(x_t[:, b, h * HIH:(h + 1) * HIH], x_sb[:])
        return x_t

    def mm2_part(pps_tiles, c, m1_c, half):
        # P[o] accumulation for one c: 16 matmuls of N=256, two o's per
        # PSUM tile column-block via tile_position row packing; emitted in
        # a 9-MM and a 7-MM batch so they pad the two m1_ps WAR windows
        # (DVE evictions need ~1292ns, Act ~1138ns).
        # Two o-pairs share each 2KB PSUM bank. start=True marks the whole
        # bank (per touched partition range) pending-zero, so only the
        # first column block (t%2==0) starts; block 1's first write then
        # lands on pending-zero bytes and overwrites correctly.
        rng = range(0, 8) if half == 0 else range(8, 16)
        for k in rng:
            t, ph = k // 2, k % 2
            pt = pps_tiles[t // 2]
            o = 2 * t + ph
            nc.tensor.matmul(
                pt[ph * 64:(ph + 1) * 64, t % 2, :],
                w_bf[:, c, o, :],
                m1_c[:, :, o * NO:(o + 1) * NO],
                start=(c == 0 and t % 2 == 0),
                stop=(c == HI - 1),
                tile_position=(0, ph * 64),
                skip_group_check=True,
            )

    def mm1(x_t, c, b, m1_c):
        m1_ps = m1ps_pool.tile([NI, OQ], F32, tag="m1ps")
        lhsT = x_t[:, b, c, :]
        for h in range(2):
            nc.tensor.matmul(
                m1_ps[:, h * 512:(h + 1) * 512],
                lhsT,
                w_bf[:, c, h * 8:(h + 1) * 8, :],
                start=True,
                stop=True,
            )
        # One eviction copy per batch, alternating DVE/Act. Any finer split
        # (even h-aligned staggered halves) adds ~400ns effective per-copy
        # overhead and measured worse in every arrangement tried.
        if b % 2 == 0:
            nc.vector.tensor_copy(m1_c[:, b, :], m1_ps[:, :])
        else:
            nc.scalar.copy(m1_c[:, b, :], m1_ps[:, :])

    def evict_p_pair(p_sb, pps_tiles, b0, i, eng, dma_qs=None):
        # Copy accumulator pair-tile i (o-pairs 2i, 2i+1) to SBUF, then
        # one DMA per o-pair. GPSIMD can't read PSUM on HW, so these ride
        # DVE/Act; callers stage pairs across c=0/c=1 of the next group to
        # stay inside the per-c engine budgets. For the last group (p_sb
        # None) no mm2 follows, so DMA straight from PSUM and skip the
        # staging copy.
        if p_sb is not None:
            if eng == "v":
                nc.vector.tensor_copy(p_sb[:, 2 * i:2 * i + 2, :], pps_tiles[i][:])
            else:
                nc.scalar.copy(p_sb[:, 2 * i:2 * i + 2, :], pps_tiles[i][:])
        for k in range(2):
            t = 2 * i + k
            q = dma_qs[k] if dma_qs else nc.sync
            src = p_sb[:, t, :] if p_sb is not None else pps_tiles[i][:, k, :]
            q.dma_start(
                out=p_out[b0:b0 + BG, 2 * t:2 * t + 2].rearrange(
                    "b o p q -> (o p) b q"
                ),
                in_=src,
            )

    x_tiles = {0: x_load(0, fine_b0=(w_load_rest is not None))}
    if w_load_rest is not None:
        for c in range(1, HI):
            w_load_rest(c)
    prev = None  # [pps_tiles, m1_tile_of_c7, b0, g, p_sb] pending mm2(c=7) + evict

    for g in range(NG):
        b0 = g * BG
        if g + 1 < NG:
            x_tiles[g + 1] = x_load(g + 1)
        x_t = x_tiles.pop(g)

        pps_tiles = [
            pps_pool.tile([NI, 2, BG * NO], F32, tag="pps", name=f"pps{g}_{i}")
            for i in range(4)
        ]

        m1_prev = None
        for c in range(HI):
            m1_c = m1pool.tile([NI, BG, OQ], BF16, tag="m1")
            mm1(x_t, c, 0, m1_c)
            mm1(x_t, c, 1, m1_c)
            if c > 0:
                mm2_part(pps_tiles, c - 1, m1_prev, 0)
            elif prev is not None:
                mm2_part(prev[0], HI - 1, prev[1], 0)
            if c == 0 and prev is not None:
                # P eviction of the previous group, pair tiles 0,1 (o 0..3):
                # emitted here (after their stops in mm2(c7) h0) so they sit
                # between m1-evict halves in the Act FIFO without starving
                # the WAR chains.
                ppp, m1p, b0p, gp, _ = prev
                p_sb = ppool.tile([NI, HO // 2, BG * NO], F32, tag="psb",
                                  name=f"psb{gp}")
                prev[4] = p_sb
                evict_p_pair(p_sb, ppp, b0p, 0, "v")
                evict_p_pair(p_sb, ppp, b0p, 1, "a")
            elif c == 1 and prev is not None:
                ppp, m1p, b0p, gp, p_sb = prev
                evict_p_pair(p_sb, ppp, b0p, 2, "v")
                evict_p_pair(p_sb, ppp, b0p, 3, "a")
            mm1(x_t, c, 2, m1_c)
            mm1(x_t, c, 3, m1_c)
            if c > 0:
                mm2_part(pps_tiles, c - 1, m1_prev, 1)
                if c == 1:
                    prev = None
            elif prev is not None:
                mm2_part(prev[0], HI - 1, prev[1], 1)
            m1_prev = m1_c

        prev = [pps_tiles, m1_prev, b0, g, None]

    ppp, m1p, b0p, gp, _ = prev
    mm2_part(ppp, HI - 1, m1p, 0)
    mm2_part(ppp, HI - 1, m1p, 1)
    p_sb = ppool.tile([NI, HO // 2, BG * NO], F32, tag="psb", name=f"psb{gp}")
    for i in range(4):
        evict_p_pair(p_sb, ppp, b0p, i, "v" if i % 2 == 0 else "a",
                     dma_qs=(nc.sync, nc.scalar))


def kernel(x: np.ndarray, W: np.ndarray) -> np.ndarray:
    from concourse.bass_utils import run_bass_kernel_spmd

    x = np.ascontiguousarray(x, dtype=np.float32)
    W = np.ascontiguousarray(W, dtype=np.float32)

    if "nc" not in _NC_CACHE:
        _NC_CACHE["nc"] = build_nc()
    nc = _NC_CACHE["nc"]

    in_maps = [
        {"x": x[i * B : (i + 1) * B], "W": W} for i in range(NCORES)
    ]
    res = run_bass_kernel_spmd(nc, in_maps, list(range(NCORES)))
    out = np.concatenate([res.results[i]["P"] for i in range(NCORES)], axis=0)
    return out
